# revision 13
# baseline (speedup 1.0000x reference)
"""HELMo encoder (bi-GRU over 3 steps + MHA + classifier) on 8 trn2 cores.

Data-parallel over batch (8192 -> 8 x 1024). Per core, one Bass/Tile kernel:
  A) fused GRU: input and hidden projections accumulate into shared PSUM
     (k = [x; h_prev] against W_cat = [W_ih.T; W_hh.T]), gates on ACT/DVE,
     feature-major layout (features on partitions, batch on free dim).
  B) Q/K/V projections emitted batch-major directly by using hs chunks as the
     matmul stationary operand (out[b, d_out] = hs[d_in, b].T @ W.T[d_in, d_out]).
  C) attention combine on DVE: per-head segment-reduce logits, softmax,
     then ctx_sum = sum_tk (sum_tq w[h,tq,tk]) * V[tk]  (Wo folded over t).
  D) att = ctx_sum @ Wo.T back in feature-major via PE transposes of ctx_sum.
  E) o = att.T @ W_out.T + b_out, softmax over 7 classes.

All big matmuls run in float32r (~1.3e-4 rel err, full PE rate).
"""

import sys

sys.path.insert(0, "/opt/trn_rl_repo")

import numpy as np

import concourse.bacc as bacc
import concourse.bass as bass
import concourse.mybir as mybir
import concourse.tile as tile
from concourse.masks import make_identity

dt = mybir.dt
AF = mybir.ActivationFunctionType
AX = mybir.AxisListType

N_CORES = 8
B = 8192
B_LOC = B // N_CORES          # 1024
I = 1024
H = 1024
D = 2 * H                     # 2048
NH = 16
HD = 128
S = 3
C = 7
P = 128
HJT = H // P                  # 8 jtiles per gate
KC_D = D // P                 # 16

_CACHE = {}


def _r3(ap, pat, **kw):
    return ap.rearrange(pat, **kw)


def build_nc():
    nc = bacc.Bacc("TRN2", target_bir_lowering=False, debug=False,
                   num_devices=N_CORES)

    f32, f32r = dt.float32, dt.float32r
    xt = nc.dram_tensor("xt", [S, I, B_LOC], f32r, kind="ExternalInput")
    wcat = {d: nc.dram_tensor(f"wcat_{d}", [2 * H, 3 * H], f32r, kind="ExternalInput")
            for d in ("f", "b")}
    wq = nc.dram_tensor("wq", [D, D], f32r, kind="ExternalInput")
    wk = nc.dram_tensor("wk", [D, D], f32r, kind="ExternalInput")
    wv = nc.dram_tensor("wv", [D, D], f32r, kind="ExternalInput")
    wo = nc.dram_tensor("wo", [D, D], f32r, kind="ExternalInput")
    wout = nc.dram_tensor("wout", [D, C], f32, kind="ExternalInput")
    brz = {d: nc.dram_tensor(f"brz_{d}", [2 * H, 1], f32, kind="ExternalInput")
           for d in ("f", "b")}
    negbz = {d: nc.dram_tensor(f"negbz_{d}", [H, 1], f32, kind="ExternalInput")
             for d in ("f", "b")}
    bnih = {d: nc.dram_tensor(f"bnih_{d}", [H, 1], f32, kind="ExternalInput")
            for d in ("f", "b")}
    bnhh = {d: nc.dram_tensor(f"bnhh_{d}", [H, 1], f32, kind="ExternalInput")
            for d in ("f", "b")}
    bout = nc.dram_tensor("bout", [1, C], f32, kind="ExternalInput")
    o_out = nc.dram_tensor("o_out", [B_LOC, C], f32, kind="ExternalOutput")
    sm_out = nc.dram_tensor("sm_out", [B_LOC, C], f32, kind="ExternalOutput")

    with tile.TileContext(nc) as tc:
        with tc.tile_pool(name="dram", bufs=1, space="DRAM") as dram:
            hs = dram.tile([S, D, B_LOC], f32r)
            qs = dram.tile([S, B_LOC, D], dt.bfloat16)
            ks = dram.tile([S, B_LOC, D], dt.bfloat16)
            vs = dram.tile([S, B_LOC, D], f32)
            ctxT = dram.tile([D, B_LOC], f32r)

            # ---------------- Phase A: GRU ----------------
            with (tc.tile_pool(name="ga_const", bufs=1) as cpool,
                  tc.tile_pool(name="ga_x", bufs=1) as xpool,
                  tc.tile_pool(name="ga_h", bufs=3) as hpool,
                  tc.tile_pool(name="ga_w", bufs=2) as wpool,
                  tc.tile_pool(name="ga_g", bufs=2) as gpool,
                  tc.tile_pool(name="ga_t", bufs=3) as tpool,
                  tc.tile_pool(name="ga_ps", bufs=2, space="PSUM") as pps):
                bias = {}
                for d in ("f", "b"):
                    t_brz = cpool.tile([P, 2 * HJT, 1], f32, tag=f"brz{d}")
                    nc.sync.dma_start(t_brz[:], _r3(brz[d][:], "(c k) o -> k c o", k=P))
                    t_nbz = cpool.tile([P, HJT, 1], f32, tag=f"nbz{d}")
                    nc.sync.dma_start(t_nbz[:], _r3(negbz[d][:], "(c k) o -> k c o", k=P))
                    t_bni = cpool.tile([P, HJT, 1], f32, tag=f"bni{d}")
                    nc.sync.dma_start(t_bni[:], _r3(bnih[d][:], "(c k) o -> k c o", k=P))
                    t_bnh = cpool.tile([P, HJT, 1], f32, tag=f"bnh{d}")
                    nc.sync.dma_start(t_bnh[:], _r3(bnhh[d][:], "(c k) o -> k c o", k=P))
                    bias[d] = (t_brz, t_nbz, t_bni, t_bnh)

                order = [(0, "f", 0), (0, "b", 2), (1, "f", 1),
                         (1, "b", 1), (2, "f", 2), (2, "b", 0)]
                h_cur = {"f": None, "b": None}
                for step, d, t in order:
                    t_brz, t_nbz, t_bni, t_bnh = bias[d]
                    first = step == 0
                    x_t = xpool.tile([P, HJT, B_LOC], f32r, tag="x")
                    nc.sync.dma_start(x_t[:], _r3(xt[t], "(c k) b -> k c b", k=P))
                    h_prev = h_cur[d]
                    h_new = hpool.tile([P, HJT, B_LOC], f32r, tag="h")
                    for j in range(HJT):
                        wslice = {}
                        for gi_, gname in ((j, "wr"), (HJT + j, "wz"), (2 * HJT + j, "wn")):
                            ws = wpool.tile([P, 2 * HJT, P], f32r, tag=gname)
                            nc.sync.dma_start(
                                ws[:],
                                _r3(wcat[d][:, gi_ * P:(gi_ + 1) * P],
                                    "(c k) m -> k c m", k=P))
                            wslice[gname] = ws
                        for bt in range(2):
                            bs = slice(bt * 512, (bt + 1) * 512)
                            nk = HJT if first else 2 * HJT

                            def mm_acc(ptile, ws):
                                for c in range(nk):
                                    rhs = (x_t[:, c, bs] if c < HJT
                                           else h_prev[:, c - HJT, bs])
                                    nc.tensor.matmul(ptile[:], ws[:, c, :], rhs,
                                                     start=(c == 0),
                                                     stop=(c == nk - 1))

                            pr = pps.tile([P, 512], f32, tag="pr")
                            mm_acc(pr, wslice["wr"])
                            pz = pps.tile([P, 512], f32, tag="pz")
                            mm_acc(pz, wslice["wz"])
                            pgi = pps.tile([P, 512], f32, tag="pgi")
                            for c in range(HJT):
                                nc.tensor.matmul(pgi[:], wslice["wn"][:, c, :],
                                                 x_t[:, c, bs],
                                                 start=(c == 0), stop=(c == HJT - 1))
                            r_sb = gpool.tile([P, 512], f32, tag="r")
                            nc.scalar.activation(r_sb[:], pr[:], AF.Sigmoid,
                                                 bias=t_brz[:, j, :])
                            n_sb = gpool.tile([P, 512], f32, tag="n")
                            if first:
                                zc = gpool.tile([P, 512], f32, tag="z")
                                nc.scalar.activation(zc[:], pz[:], AF.Sigmoid,
                                                     bias=t_nbz[:, j, :], scale=-1.0)
                                nc.scalar.activation(n_sb[:], pgi[:], AF.Tanh,
                                                     bias=t_bni[:, j, :])
                                nc.vector.tensor_mul(h_new[:, j, bs], zc[:], n_sb[:])
                            else:
                                z_sb = gpool.tile([P, 512], f32, tag="z")
                                nc.scalar.activation(z_sb[:], pz[:], AF.Sigmoid,
                                                     bias=t_brz[:, HJT + j, :])
                                pgh = pps.tile([P, 512], f32, tag="pgh")
                                for c in range(HJT, 2 * HJT):
                                    nc.tensor.matmul(pgh[:], wslice["wn"][:, c, :],
                                                     h_prev[:, c - HJT, bs],
                                                     start=(c == HJT),
                                                     stop=(c == 2 * HJT - 1))
                                t1 = tpool.tile([P, 512], f32, tag="tmp")
                                nc.vector.tensor_scalar_add(t1[:], pgh[:],
                                                            t_bnh[:, j, :])
                                t2 = tpool.tile([P, 512], f32, tag="tmp")
                                nc.vector.tensor_mul(t2[:], r_sb[:], t1[:])
                                t3 = tpool.tile([P, 512], f32, tag="tmp")
                                nc.vector.tensor_add(t3[:], pgi[:], t2[:])
                                nc.scalar.activation(n_sb[:], t3[:], AF.Tanh,
                                                     bias=t_bni[:, j, :])
                                t4 = tpool.tile([P, 512], f32, tag="tmp")
                                nc.vector.tensor_sub(t4[:], h_prev[:, j, bs], n_sb[:])
                                t5 = tpool.tile([P, 512], f32, tag="tmp")
                                nc.vector.tensor_mul(t5[:], z_sb[:], t4[:])
                                nc.vector.tensor_add(h_new[:, j, bs], t5[:], n_sb[:])
                            row = (0 if d == "f" else H) + j * P
                            nc.sync.dma_start(hs[t, row:row + P, bs],
                                              h_new[:, j, bs])
                    h_cur[d] = h_new

            # ---------------- Phase B: Q/K/V projections ----------------
            for wsrc, dst, odt in ((wq, qs, dt.bfloat16), (wk, ks, dt.bfloat16),
                                   (wv, vs, f32)):
                with (tc.tile_pool(name="gb_w", bufs=1) as wbpool,
                      tc.tile_pool(name="gb_s", bufs=2) as spool,
                      tc.tile_pool(name="gb_o", bufs=2) as opool,
                      tc.tile_pool(name="gb_ps", bufs=8, space="PSUM") as pps):
                    wt = wbpool.tile([P, KC_D, D], f32r, tag="wproj")
                    nc.sync.dma_start(wt[:], _r3(wsrc[:], "(c k) n -> k c n", k=P))
                    for t in range(S):
                        for btile in range(HJT):
                            hst = spool.tile([P, KC_D, P], f32r, tag="hst")
                            nc.sync.dma_start(
                                hst[:],
                                _r3(hs[t][:, btile * P:(btile + 1) * P],
                                    "(c k) b -> k c b", k=P))
                            po = [pps.tile([P, 512], f32, tag="pqkv",
                                           name=f"pqkv_{t}_{btile}_{i}")
                                  for i in range(4)]
                            for c in range(KC_D):
                                for do_ in range(4):
                                    nc.tensor.matmul(
                                        po[do_][:], hst[:, c, :],
                                        wt[:, c, do_ * 512:(do_ + 1) * 512],
                                        start=(c == 0), stop=(c == KC_D - 1))
                            osb = opool.tile([P, D], odt, tag="qkvout",
                                             name=f"qkvout_{t}_{btile}")
                            for do_ in range(4):
                                nc.scalar.copy(osb[:, do_ * 512:(do_ + 1) * 512],
                                               po[do_][:])
                            nc.sync.dma_start(
                                dst[t][btile * P:(btile + 1) * P, :], osb[:])

            # ---------------- Phase C: attention combine ----------------
            with (tc.tile_pool(name="gc_inqk", bufs=1) as inqk_pool,
                  tc.tile_pool(name="gc_inv", bufs=2) as inv_pool,
                  tc.tile_pool(name="gc_w", bufs=2) as wkpool,
                  tc.tile_pool(name="gc_t", bufs=1) as tmpool,
                  tc.tile_pool(name="gc_c", bufs=2) as ctxpool,
                  tc.tile_pool(name="gc_1", bufs=1) as one_pool,
                  tc.tile_pool(name="gc_ps", bufs=2, space="PSUM") as pps):
                ident = one_pool.tile([P, P], f32, tag="ident")
                make_identity(nc, ident[:])
                for btile in range(HJT):
                    bsl = slice(btile * P, (btile + 1) * P)
                    qt, kt, vt = [], [], []
                    for t in range(S):
                        for src_, lst, nm, pool_ in (
                                (qs, qt, "q", inqk_pool), (ks, kt, "k", inqk_pool),
                                (vs, vt, "v", inv_pool)):
                            tl = pool_.tile([P, D],
                                            dt.bfloat16 if nm in ("q", "k") else f32,
                                            tag=f"{nm}{t}",
                                            name=f"{nm}{t}_{btile}")
                            nc.sync.dma_start(tl[:], src_[t][bsl, :])
                            lst.append(tl)
                    L = wkpool.tile([P, NH, S, S], f32, tag="L")
                    prod_tag = 0
                    for tq in range(S):
                        for tk in range(S):
                            pr_ = wkpool.tile([P, D], dt.bfloat16, tag="prod",
                                              name=f"prod_{btile}_{tq}_{tk}")
                            nc.vector.tensor_mul(pr_[:], qt[tq][:], kt[tk][:])
                            nc.vector.reduce_sum(
                                L[:, :, tq, tk],
                                _r3(pr_[:], "p (h e) -> p h e", h=NH), axis=AX.X)
                            prod_tag += 1
                    M = wkpool.tile([P, NH, S], f32, tag="M")
                    nc.vector.reduce_max(M[:], L[:], axis=AX.X)
                    E = wkpool.tile([P, NH, S, S], f32, tag="E")
                    nc.vector.tensor_sub(E[:], L[:],
                                         M[:, :, :, None].broadcast_to([P, NH, S, S]))
                    E2 = wkpool.tile([P, NH, S, S], f32, tag="E2")
                    nc.scalar.activation(E2[:], E[:], AF.Exp)
                    Ssum = wkpool.tile([P, NH, S], f32, tag="Ssum")
                    nc.vector.reduce_sum(Ssum[:], E2[:], axis=AX.X)
                    Rs = wkpool.tile([P, NH, S], f32, tag="Rs")
                    nc.vector.reciprocal(Rs[:], Ssum[:])
                    Wn = wkpool.tile([P, NH, S, S], f32, tag="Wn")
                    nc.vector.tensor_mul(Wn[:], E2[:],
                                         Rs[:, :, :, None].broadcast_to([P, NH, S, S]))
                    wsum = wkpool.tile([P, NH, S], f32, tag="wsum")
                    nc.vector.reduce_sum(wsum[:], _r3(Wn[:], "p h q k -> p h k q"),
                                         axis=AX.X)
                    ctx = ctxpool.tile([P, D], f32, tag="ctx")
                    tm0 = tmpool.tile([P, D], f32, tag="ctmp0")
                    nc.vector.tensor_mul(
                        _r3(tm0[:], "p (h e) -> p h e", h=NH),
                        _r3(vt[0][:], "p (h e) -> p h e", h=NH),
                        wsum[:, :, 0][:, :, None].broadcast_to([P, NH, HD]))
                    tm1 = tmpool.tile([P, D], f32, tag="ctmp1")
                    nc.vector.tensor_mul(
                        _r3(tm1[:], "p (h e) -> p h e", h=NH),
                        _r3(vt[1][:], "p (h e) -> p h e", h=NH),
                        wsum[:, :, 1][:, :, None].broadcast_to([P, NH, HD]))
                    nc.vector.tensor_add(tm0[:], tm0[:], tm1[:])
                    nc.vector.tensor_mul(
                        _r3(tm1[:], "p (h e) -> p h e", h=NH),
                        _r3(vt[2][:], "p (h e) -> p h e", h=NH),
                        wsum[:, :, 2][:, :, None].broadcast_to([P, NH, HD]))
                    nc.vector.tensor_add(ctx[:], tm0[:], tm1[:])
                    for c in range(KC_D):
                        pt = pps.tile([P, P], f32, tag="ptr")
                        nc.tensor.transpose(pt[:], ctx[:, c * P:(c + 1) * P],
                                            ident[:])
                        st = ctxpool.tile([P, P], f32r, tag="sttr")
                        nc.vector.tensor_copy(st[:], pt[:])
                        nc.sync.dma_start(ctxT[c * P:(c + 1) * P, bsl], st[:])

            # ---------------- Phase D: att = ctx_sum @ Wo.T ----------------
            att_h = None
            with (tc.tile_pool(name="gd_c", bufs=2) as cpool2,
                  tc.tile_pool(name="gd_w", bufs=2) as wpool2,
                  tc.tile_pool(name="gd_a", bufs=1) as apool,
                  tc.tile_pool(name="gd_ps", bufs=4, space="PSUM") as pps):
                att_h = apool.tile([P, KC_D, B_LOC], f32, tag="att")
                for bt in range(2):
                    bs = slice(bt * 512, (bt + 1) * 512)
                    ct = cpool2.tile([P, KC_D, 512], f32r, tag="ctxT")
                    nc.sync.dma_start(ct[:], _r3(ctxT[:, bs], "(c k) b -> k c b", k=P))
                    for jt in range(KC_D):
                        wos = wpool2.tile([P, KC_D, P], f32r, tag="wos")
                        nc.sync.dma_start(
                            wos[:],
                            _r3(wo[:, jt * P:(jt + 1) * P], "(c k) m -> k c m", k=P))
                        pw = pps.tile([P, 512], f32, tag="pwo")
                        for c in range(KC_D):
                            nc.tensor.matmul(pw[:], wos[:, c, :], ct[:, c, :],
                                             start=(c == 0), stop=(c == KC_D - 1))
                        nc.vector.tensor_copy(att_h[:, jt, bs], pw[:])

                # ---------------- Phase E: classifier + softmax ----------------
                with (tc.tile_pool(name="ge", bufs=2) as epool,
                      tc.tile_pool(name="ge1", bufs=1) as e1pool,
                      tc.tile_pool(name="ge_ps", bufs=2, space="PSUM") as pps2):
                    wout_sb = e1pool.tile([P, KC_D, C], f32, tag="wout")
                    nc.sync.dma_start(wout_sb[:], _r3(wout[:], "(c k) n -> k c n", k=P))
                    bout_sb = e1pool.tile([P, C], f32, tag="bout")
                    nc.sync.dma_start(bout_sb[:], bout[:].to_broadcast([P, C]))
                    for btile in range(HJT):
                        bsl = slice(btile * P, (btile + 1) * P)
                        pf = pps2.tile([P, C], f32, tag="pf")
                        for c in range(KC_D):
                            nc.tensor.matmul(pf[:], att_h[:, c, bsl],
                                             wout_sb[:, c, :],
                                             start=(c == 0), stop=(c == KC_D - 1))
                        o_sb = epool.tile([P, C], f32, tag="osb")
                        nc.vector.tensor_add(o_sb[:], pf[:], bout_sb[:])
                        nc.sync.dma_start(o_out[bsl, :], o_sb[:])
                        mx = epool.tile([P, 1], f32, tag="mx")
                        nc.vector.reduce_max(mx[:], o_sb[:], axis=AX.X)
                        nmx = epool.tile([P, 1], f32, tag="nmx")
                        nc.vector.tensor_scalar_mul(nmx[:], mx[:], -1.0)
                        esb = epool.tile([P, C], f32, tag="esb")
                        nc.scalar.activation(esb[:], o_sb[:], AF.Exp, bias=nmx[:])
                        ssb = epool.tile([P, 1], f32, tag="ssb")
                        nc.vector.reduce_sum(ssb[:], esb[:], axis=AX.X)
                        rsb = epool.tile([P, 1], f32, tag="rsb")
                        nc.vector.reciprocal(rsb[:], ssb[:])
                        smsb = epool.tile([P, C], f32, tag="smsb")
                        nc.vector.tensor_mul(smsb[:], esb[:],
                                             rsb[:].broadcast_to([P, C]))
                        nc.sync.dma_start(sm_out[bsl, :], smsb[:])

    nc.compile()
    return nc


def _prep_inputs(inputs):
    f32 = np.float32
    xs = np.stack([np.asarray(inputs["x1"], f32), np.asarray(inputs["x2"], f32),
                   np.asarray(inputs["x3"], f32)])  # (3, B, I)
    shared = {}
    for d in ("f", "b"):
        wih = np.asarray(inputs[f"W_ih_{d}"], f32)
        whh = np.asarray(inputs[f"W_hh_{d}"], f32)
        bih = np.asarray(inputs[f"b_ih_{d}"], f32)
        bhh = np.asarray(inputs[f"b_hh_{d}"], f32)
        shared[f"wcat_{d}"] = np.ascontiguousarray(
            np.concatenate([wih.T, whh.T], axis=0))
        bsum = bih + bhh
        shared[f"brz_{d}"] = np.ascontiguousarray(bsum[:2 * H, None])
        shared[f"negbz_{d}"] = np.ascontiguousarray(-bsum[H:2 * H, None])
        shared[f"bnih_{d}"] = np.ascontiguousarray(bih[2 * H:, None])
        shared[f"bnhh_{d}"] = np.ascontiguousarray(bhh[2 * H:, None])
    shared["wq"] = np.ascontiguousarray(
        np.asarray(inputs["Wq"], f32).T * (HD ** -0.5))
    shared["wk"] = np.ascontiguousarray(np.asarray(inputs["Wk"], f32).T)
    shared["wv"] = np.ascontiguousarray(np.asarray(inputs["Wv"], f32).T)
    shared["wo"] = np.ascontiguousarray(np.asarray(inputs["Wo"], f32).T)
    shared["wout"] = np.ascontiguousarray(np.asarray(inputs["W_out"], f32).T)
    shared["bout"] = np.ascontiguousarray(np.asarray(inputs["b_out"], f32)[None, :])

    in_maps = []
    for c in range(N_CORES):
        rows = slice(c * B_LOC, (c + 1) * B_LOC)
        m = dict(shared)
        m["xt"] = np.ascontiguousarray(xs[:, rows, :].transpose(0, 2, 1))
        in_maps.append(m)
    return in_maps


def _get_nc():
    if "nc" not in _CACHE:
        _CACHE["nc"] = build_nc()
    return _CACHE["nc"]


def kernel(**inputs):
    from concourse.bass_utils import run_bass_kernel_spmd

    nc = _get_nc()
    in_maps = _prep_inputs(inputs)
    res = run_bass_kernel_spmd(nc, in_maps, core_ids=list(range(N_CORES)))
    o = np.concatenate([res.results[c]["o_out"] for c in range(N_CORES)], axis=0)
    sm = np.concatenate([res.results[c]["sm_out"] for c in range(N_CORES)], axis=0)
    return o, sm


# revision 20
# speedup vs baseline: 1.0781x; 1.0781x over previous
"""HELMo encoder (bi-GRU over 3 steps + MHA + classifier) on 8 trn2 cores.

Data-parallel over batch (8192 -> 8 x 1024). Per core, one Bass/Tile kernel:
  A) fused GRU: input and hidden projections accumulate into shared PSUM
     (k = [x; h_prev] against W_cat = [W_ih.T; W_hh.T]), gates on ACT/DVE,
     feature-major layout (features on partitions, batch on free dim).
  B) Q/K/V projections emitted batch-major directly by using hs chunks as the
     matmul stationary operand (out[b, d_out] = hs[d_in, b].T @ W.T[d_in, d_out]).
  C) attention combine on DVE: per-head segment-reduce logits, softmax,
     then ctx_sum = sum_tk (sum_tq w[h,tq,tk]) * V[tk]  (Wo folded over t).
  D) att = ctx_sum @ Wo.T back in feature-major via PE transposes of ctx_sum.
  E) o = att.T @ W_out.T + b_out, softmax over 7 classes.

All big matmuls run in float32r (~1.3e-4 rel err, full PE rate).
"""

import sys

sys.path.insert(0, "/opt/trn_rl_repo")

import numpy as np

import concourse.bacc as bacc
import concourse.bass as bass
import concourse.mybir as mybir
import concourse.tile as tile
from concourse.masks import make_identity

dt = mybir.dt
AF = mybir.ActivationFunctionType
AX = mybir.AxisListType

N_CORES = 8
B = 8192
B_LOC = B // N_CORES          # 1024
I = 1024
H = 1024
D = 2 * H                     # 2048
NH = 16
HD = 128
S = 3
C = 7
P = 128
HJT = H // P                  # 8 jtiles per gate
KC_D = D // P                 # 16

_CACHE = {}


def _r3(ap, pat, **kw):
    return ap.rearrange(pat, **kw)


def build_nc(phases="abcde", reps=1):
    nc = bacc.Bacc("TRN2", target_bir_lowering=False, debug=False,
                   num_devices=N_CORES)

    f32, f32r = dt.float32, dt.float32r
    xt = nc.dram_tensor("xt", [S, I, B_LOC], f32r, kind="ExternalInput")
    wcat = {d: nc.dram_tensor(f"wcat_{d}", [2 * H, 3 * H], f32r, kind="ExternalInput")
            for d in ("f", "b")}
    wq = nc.dram_tensor("wq", [D, D], f32r, kind="ExternalInput")
    wk = nc.dram_tensor("wk", [D, D], f32r, kind="ExternalInput")
    wv = nc.dram_tensor("wv", [D, D], f32r, kind="ExternalInput")
    wo = nc.dram_tensor("wo", [D, D], f32r, kind="ExternalInput")
    wout = nc.dram_tensor("wout", [D, C], f32, kind="ExternalInput")
    brz = {d: nc.dram_tensor(f"brz_{d}", [2 * H, 1], f32, kind="ExternalInput")
           for d in ("f", "b")}
    negbz = {d: nc.dram_tensor(f"negbz_{d}", [H, 1], f32, kind="ExternalInput")
             for d in ("f", "b")}
    bnih = {d: nc.dram_tensor(f"bnih_{d}", [H, 1], f32, kind="ExternalInput")
            for d in ("f", "b")}
    bnhh = {d: nc.dram_tensor(f"bnhh_{d}", [H, 1], f32, kind="ExternalInput")
            for d in ("f", "b")}
    bout = nc.dram_tensor("bout", [1, C], f32, kind="ExternalInput")
    o_out = nc.dram_tensor("o_out", [B_LOC, C], f32, kind="ExternalOutput")
    sm_out = nc.dram_tensor("sm_out", [B_LOC, C], f32, kind="ExternalOutput")

    with tile.TileContext(nc) as tc:
      for _rep in range(reps):
        with tc.tile_pool(name="dram", bufs=1, space="DRAM") as dram:
            hs = dram.tile([S, D, B_LOC], f32r)
            qs = dram.tile([S, B_LOC, D], dt.bfloat16)
            ks = dram.tile([S, B_LOC, D], dt.bfloat16)
            vs = dram.tile([S, B_LOC, D], f32)
            att_d = dram.tile([D, B_LOC], f32)

            # ---------------- Phase A: GRU ----------------
            if "a" in phases:
              with (tc.tile_pool(name="ga_const", bufs=1) as cpool,
                  tc.tile_pool(name="ga_x", bufs=1) as xpool,
                  tc.tile_pool(name="ga_h", bufs=3) as hpool,
                  tc.tile_pool(name="ga_w", bufs=2) as wpool,
                  tc.tile_pool(name="ga_g", bufs=2) as gpool,
                  tc.tile_pool(name="ga_t", bufs=3) as tpool,
                  tc.tile_pool(name="ga_ps", bufs=2, space="PSUM") as pps):
                bias = {}
                for d in ("f", "b"):
                    t_brz = cpool.tile([P, 2 * HJT, 1], f32, tag=f"brz{d}")
                    nc.sync.dma_start(t_brz[:], _r3(brz[d][:], "(c k) o -> k c o", k=P))
                    t_nbz = cpool.tile([P, HJT, 1], f32, tag=f"nbz{d}")
                    nc.sync.dma_start(t_nbz[:], _r3(negbz[d][:], "(c k) o -> k c o", k=P))
                    t_bni = cpool.tile([P, HJT, 1], f32, tag=f"bni{d}")
                    nc.sync.dma_start(t_bni[:], _r3(bnih[d][:], "(c k) o -> k c o", k=P))
                    t_bnh = cpool.tile([P, HJT, 1], f32, tag=f"bnh{d}")
                    nc.sync.dma_start(t_bnh[:], _r3(bnhh[d][:], "(c k) o -> k c o", k=P))
                    bias[d] = (t_brz, t_nbz, t_bni, t_bnh)

                order = [(0, "f", 0), (0, "b", 2), (1, "f", 1),
                         (1, "b", 1), (2, "f", 2), (2, "b", 0)]
                h_cur = {"f": None, "b": None}
                for step, d, t in order:
                    t_brz, t_nbz, t_bni, t_bnh = bias[d]
                    first = step == 0
                    x_t = xpool.tile([P, HJT, B_LOC], f32r, tag="x")
                    nc.sync.dma_start(x_t[:], _r3(xt[t], "(c k) b -> k c b", k=P))
                    h_prev = h_cur[d]
                    h_new = hpool.tile([P, HJT, B_LOC], f32r, tag="h")
                    for j in range(HJT):
                        # host pre-permutes wcat columns: per j the r/z/n gate
                        # columns are adjacent -> one contiguous 384-col DMA
                        nkc = HJT if first else 2 * HJT
                        wj = wpool.tile([P, nkc, 3 * P], f32r, tag="wj",
                                        name=f"wj_{step}_{d}_{j}")
                        nc.sync.dma_start(
                            wj[:],
                            _r3(wcat[d][:nkc * P, j * 3 * P:(j + 1) * 3 * P],
                                "(c k) m -> k c m", k=P))
                        wslice = {"wr": wj[:, :, 0:P], "wz": wj[:, :, P:2 * P],
                                  "wn": wj[:, :, 2 * P:3 * P]}
                        for bt in range(2):
                            bs = slice(bt * 512, (bt + 1) * 512)
                            nk = HJT if first else 2 * HJT

                            def mm_acc(ptile, ws):
                                for c in range(nk):
                                    rhs = (x_t[:, c, bs] if c < HJT
                                           else h_prev[:, c - HJT, bs])
                                    nc.tensor.matmul(ptile[:], ws[:, c, :], rhs,
                                                     start=(c == 0),
                                                     stop=(c == nk - 1))

                            pr = pps.tile([P, 512], f32, tag="pr")
                            mm_acc(pr, wslice["wr"])
                            pz = pps.tile([P, 512], f32, tag="pz")
                            mm_acc(pz, wslice["wz"])
                            pgi = pps.tile([P, 512], f32, tag="pgi")
                            for c in range(HJT):
                                nc.tensor.matmul(pgi[:], wslice["wn"][:, c, :],
                                                 x_t[:, c, bs],
                                                 start=(c == 0), stop=(c == HJT - 1))
                            r_sb = gpool.tile([P, 512], f32, tag="r")
                            nc.scalar.activation(r_sb[:], pr[:], AF.Sigmoid,
                                                 bias=t_brz[:, j, :])
                            n_sb = gpool.tile([P, 512], f32, tag="n")
                            if first:
                                zc = gpool.tile([P, 512], f32, tag="z")
                                nc.scalar.activation(zc[:], pz[:], AF.Sigmoid,
                                                     bias=t_nbz[:, j, :], scale=-1.0)
                                nc.scalar.activation(n_sb[:], pgi[:], AF.Tanh,
                                                     bias=t_bni[:, j, :])
                                nc.vector.tensor_mul(h_new[:, j, bs], zc[:], n_sb[:])
                            else:
                                z_sb = gpool.tile([P, 512], f32, tag="z")
                                nc.scalar.activation(z_sb[:], pz[:], AF.Sigmoid,
                                                     bias=t_brz[:, HJT + j, :])
                                pgh = pps.tile([P, 512], f32, tag="pgh")
                                for c in range(HJT, 2 * HJT):
                                    nc.tensor.matmul(pgh[:], wslice["wn"][:, c, :],
                                                     h_prev[:, c - HJT, bs],
                                                     start=(c == HJT),
                                                     stop=(c == 2 * HJT - 1))
                                t1 = tpool.tile([P, 512], f32, tag="tmp")
                                nc.vector.tensor_scalar_add(t1[:], pgh[:],
                                                            t_bnh[:, j, :])
                                t2 = tpool.tile([P, 512], f32, tag="tmp")
                                nc.vector.tensor_mul(t2[:], r_sb[:], t1[:])
                                t3 = tpool.tile([P, 512], f32, tag="tmp")
                                nc.vector.tensor_add(t3[:], pgi[:], t2[:])
                                nc.scalar.activation(n_sb[:], t3[:], AF.Tanh,
                                                     bias=t_bni[:, j, :])
                                t4 = tpool.tile([P, 512], f32, tag="tmp")
                                nc.vector.tensor_sub(t4[:], h_prev[:, j, bs], n_sb[:])
                                t5 = tpool.tile([P, 512], f32, tag="tmp")
                                nc.vector.tensor_mul(t5[:], z_sb[:], t4[:])
                                nc.vector.tensor_add(h_new[:, j, bs], t5[:], n_sb[:])
                            row = (0 if d == "f" else H) + j * P
                            nc.sync.dma_start(hs[t, row:row + P, bs],
                                              h_new[:, j, bs])
                    h_cur[d] = h_new

            # ---------------- Phase B: Q/K/V projections ----------------
            if "b" in phases:
              for wsrc, dst, odt in ((wq, qs, dt.bfloat16), (wk, ks, dt.bfloat16),
                                   (wv, vs, f32)):
                with (tc.tile_pool(name="gb_w", bufs=1) as wbpool,
                      tc.tile_pool(name="gb_s", bufs=2) as spool,
                      tc.tile_pool(name="gb_o", bufs=2) as opool,
                      tc.tile_pool(name="gb_ps", bufs=8, space="PSUM") as pps):
                    wt = wbpool.tile([P, KC_D, D], f32r, tag="wproj")
                    nc.sync.dma_start(wt[:], _r3(wsrc[:], "(c k) n -> k c n", k=P))
                    for t in range(S):
                        for btile in range(HJT):
                            hst = spool.tile([P, KC_D, P], f32r, tag="hst")
                            nc.sync.dma_start(
                                hst[:],
                                _r3(hs[t][:, btile * P:(btile + 1) * P],
                                    "(c k) b -> k c b", k=P))
                            po = [pps.tile([P, 512], f32, tag="pqkv",
                                           name=f"pqkv_{t}_{btile}_{i}")
                                  for i in range(4)]
                            for c in range(KC_D):
                                for do_ in range(4):
                                    nc.tensor.matmul(
                                        po[do_][:], hst[:, c, :],
                                        wt[:, c, do_ * 512:(do_ + 1) * 512],
                                        start=(c == 0), stop=(c == KC_D - 1))
                            osb = opool.tile([P, D], odt, tag="qkvout",
                                             name=f"qkvout_{t}_{btile}")
                            for do_ in range(4):
                                nc.scalar.copy(osb[:, do_ * 512:(do_ + 1) * 512],
                                               po[do_][:])
                            nc.sync.dma_start(
                                dst[t][btile * P:(btile + 1) * P, :], osb[:])

            # ---------------- Phase C: attention combine ----------------
            if "c" in phases:
              with (tc.tile_pool(name="gc_inqk", bufs=1) as inqk_pool,
                  tc.tile_pool(name="gc_inv", bufs=1) as inv_pool,
                  tc.tile_pool(name="gc_w", bufs=2) as wkpool,
                  tc.tile_pool(name="gc_t", bufs=1) as tmpool,
                  tc.tile_pool(name="gc_c", bufs=2) as ctxpool,
                  tc.tile_pool(name="gc_m", bufs=1) as cm_pool,
                  tc.tile_pool(name="gc_wo", bufs=2) as wopool,
                  tc.tile_pool(name="gc_1", bufs=1) as one_pool,
                  tc.tile_pool(name="gc_ps", bufs=2, space="PSUM") as pps):
                ident = one_pool.tile([P, P], f32, tag="ident")
                make_identity(nc, ident[:])
                ctxm = cm_pool.tile([P, KC_D, B_LOC], f32r, tag="ctxm")
                for btile in range(HJT):
                    bsl = slice(btile * P, (btile + 1) * P)
                    qt, kt, vt = [], [], []
                    for t in range(S):
                        for src_, lst, nm, pool_ in (
                                (qs, qt, "q", inqk_pool), (ks, kt, "k", inqk_pool),
                                (vs, vt, "v", inv_pool)):
                            tl = pool_.tile([P, D],
                                            dt.bfloat16 if nm in ("q", "k") else f32,
                                            tag=f"{nm}{t}",
                                            name=f"{nm}{t}_{btile}")
                            nc.sync.dma_start(tl[:], src_[t][bsl, :])
                            lst.append(tl)
                    L = wkpool.tile([P, NH, S, S], f32, tag="L")
                    prod_tag = 0
                    for tq in range(S):
                        for tk in range(S):
                            pr_ = wkpool.tile([P, D], dt.bfloat16, tag="prod",
                                              name=f"prod_{btile}_{tq}_{tk}")
                            nc.vector.tensor_mul(pr_[:], qt[tq][:], kt[tk][:])
                            nc.vector.reduce_sum(
                                L[:, :, tq, tk],
                                _r3(pr_[:], "p (h e) -> p h e", h=NH), axis=AX.X)
                            prod_tag += 1
                    M = wkpool.tile([P, NH, S], f32, tag="M")
                    nc.vector.reduce_max(M[:], L[:], axis=AX.X)
                    E = wkpool.tile([P, NH, S, S], f32, tag="E")
                    nc.vector.tensor_sub(E[:], L[:],
                                         M[:, :, :, None].broadcast_to([P, NH, S, S]))
                    E2 = wkpool.tile([P, NH, S, S], f32, tag="E2")
                    nc.scalar.activation(E2[:], E[:], AF.Exp)
                    Ssum = wkpool.tile([P, NH, S], f32, tag="Ssum")
                    nc.vector.reduce_sum(Ssum[:], E2[:], axis=AX.X)
                    Rs = wkpool.tile([P, NH, S], f32, tag="Rs")
                    nc.vector.reciprocal(Rs[:], Ssum[:])
                    Wn = wkpool.tile([P, NH, S, S], f32, tag="Wn")
                    nc.vector.tensor_mul(Wn[:], E2[:],
                                         Rs[:, :, :, None].broadcast_to([P, NH, S, S]))
                    wsum = wkpool.tile([P, NH, S], f32, tag="wsum")
                    nc.vector.reduce_sum(wsum[:], _r3(Wn[:], "p h q k -> p h k q"),
                                         axis=AX.X)
                    ctx = ctxpool.tile([P, D], f32, tag="ctx")
                    tm0 = tmpool.tile([P, D], f32, tag="ctmp0")
                    nc.vector.tensor_mul(
                        _r3(tm0[:], "p (h e) -> p h e", h=NH),
                        _r3(vt[0][:], "p (h e) -> p h e", h=NH),
                        wsum[:, :, 0][:, :, None].broadcast_to([P, NH, HD]))
                    tm1 = tmpool.tile([P, D], f32, tag="ctmp1")
                    nc.vector.tensor_mul(
                        _r3(tm1[:], "p (h e) -> p h e", h=NH),
                        _r3(vt[1][:], "p (h e) -> p h e", h=NH),
                        wsum[:, :, 1][:, :, None].broadcast_to([P, NH, HD]))
                    nc.vector.tensor_add(tm0[:], tm0[:], tm1[:])
                    nc.vector.tensor_mul(
                        _r3(tm1[:], "p (h e) -> p h e", h=NH),
                        _r3(vt[2][:], "p (h e) -> p h e", h=NH),
                        wsum[:, :, 2][:, :, None].broadcast_to([P, NH, HD]))
                    nc.vector.tensor_add(ctx[:], tm0[:], tm1[:])
                    for c in range(KC_D):
                        pt = pps.tile([P, P], f32, tag="ptr")
                        nc.tensor.transpose(pt[:], ctx[:, c * P:(c + 1) * P],
                                            ident[:])
                        nc.vector.tensor_copy(ctxm[:, c, bsl], pt[:])
                    # after each half of the btiles, run the Wo half-pass on PE
                    # so it overlaps the DVE combine of the remaining btiles
                    if btile in (3, 7):
                        bt = btile // 4
                        bs = slice(bt * 512, (bt + 1) * 512)
                        for jt in range(KC_D):
                            wos = wopool.tile([P, KC_D, P], f32r, tag="wos",
                                              name=f"wos_{bt}_{jt}")
                            nc.sync.dma_start(
                                wos[:],
                                _r3(wo[:, jt * P:(jt + 1) * P],
                                    "(c k) m -> k c m", k=P))
                            pw = pps.tile([P, 512], f32, tag="pwo",
                                          name=f"pwo_{bt}_{jt}")
                            for c in range(KC_D):
                                nc.tensor.matmul(pw[:], wos[:, c, :],
                                                 ctxm[:, c, bs],
                                                 start=(c == 0),
                                                 stop=(c == KC_D - 1))
                            asb = ctxpool.tile([P, 512], f32, tag="asb",
                                               name=f"asb_{bt}_{jt}")
                            nc.vector.tensor_copy(asb[:], pw[:])
                            nc.sync.dma_start(att_d[jt * P:(jt + 1) * P, bs],
                                              asb[:])

            # ---------------- Phase E: classifier + softmax ----------------
            if "d" in phases:
                with (tc.tile_pool(name="ge", bufs=2) as epool,
                      tc.tile_pool(name="ge1", bufs=1) as e1pool,
                      tc.tile_pool(name="ge_ps", bufs=2, space="PSUM") as pps2):
                    wout_sb = e1pool.tile([P, KC_D, C], f32, tag="wout")
                    nc.sync.dma_start(wout_sb[:], _r3(wout[:], "(c k) n -> k c n", k=P))
                    bout_sb = e1pool.tile([P, C], f32, tag="bout")
                    nc.sync.dma_start(bout_sb[:], bout[:].to_broadcast([P, C]))
                    for btile in range(HJT):
                        bsl = slice(btile * P, (btile + 1) * P)
                        attt = epool.tile([P, KC_D, P], f32, tag="attt",
                                          name=f"attt_{btile}")
                        nc.sync.dma_start(attt[:], _r3(att_d[:, bsl],
                                                       "(c k) b -> k c b", k=P))
                        pf = pps2.tile([P, C], f32, tag="pf")
                        for c in range(KC_D):
                            nc.tensor.matmul(pf[:], attt[:, c, :],
                                             wout_sb[:, c, :],
                                             start=(c == 0), stop=(c == KC_D - 1))
                        o_sb = epool.tile([P, C], f32, tag="osb")
                        nc.vector.tensor_add(o_sb[:], pf[:], bout_sb[:])
                        nc.sync.dma_start(o_out[bsl, :], o_sb[:])
                        mx = epool.tile([P, 1], f32, tag="mx")
                        nc.vector.reduce_max(mx[:], o_sb[:], axis=AX.X)
                        nmx = epool.tile([P, 1], f32, tag="nmx")
                        nc.vector.tensor_scalar_mul(nmx[:], mx[:], -1.0)
                        esb = epool.tile([P, C], f32, tag="esb")
                        nc.scalar.activation(esb[:], o_sb[:], AF.Exp, bias=nmx[:])
                        ssb = epool.tile([P, 1], f32, tag="ssb")
                        nc.vector.reduce_sum(ssb[:], esb[:], axis=AX.X)
                        rsb = epool.tile([P, 1], f32, tag="rsb")
                        nc.vector.reciprocal(rsb[:], ssb[:])
                        smsb = epool.tile([P, C], f32, tag="smsb")
                        nc.vector.tensor_mul(smsb[:], esb[:],
                                             rsb[:].broadcast_to([P, C]))
                        nc.sync.dma_start(sm_out[bsl, :], smsb[:])

    nc.compile()
    return nc


def _prep_inputs(inputs):
    f32 = np.float32
    xs = np.stack([np.asarray(inputs["x1"], f32), np.asarray(inputs["x2"], f32),
                   np.asarray(inputs["x3"], f32)])  # (3, B, I)
    shared = {}
    for d in ("f", "b"):
        wih = np.asarray(inputs[f"W_ih_{d}"], f32)
        whh = np.asarray(inputs[f"W_hh_{d}"], f32)
        bih = np.asarray(inputs[f"b_ih_{d}"], f32)
        bhh = np.asarray(inputs[f"b_hh_{d}"], f32)
        wc = np.concatenate([wih.T, whh.T], axis=0)  # (2I, 3H)
        cols = []
        for j in range(HJT):
            for g in range(3):
                cols.append(wc[:, (g * H + j * P):(g * H + (j + 1) * P)])
        shared[f"wcat_{d}"] = np.ascontiguousarray(np.concatenate(cols, axis=1))
        bsum = bih + bhh
        shared[f"brz_{d}"] = np.ascontiguousarray(bsum[:2 * H, None])
        shared[f"negbz_{d}"] = np.ascontiguousarray(-bsum[H:2 * H, None])
        shared[f"bnih_{d}"] = np.ascontiguousarray(bih[2 * H:, None])
        shared[f"bnhh_{d}"] = np.ascontiguousarray(bhh[2 * H:, None])
    shared["wq"] = np.ascontiguousarray(
        np.asarray(inputs["Wq"], f32).T * (HD ** -0.5))
    shared["wk"] = np.ascontiguousarray(np.asarray(inputs["Wk"], f32).T)
    shared["wv"] = np.ascontiguousarray(np.asarray(inputs["Wv"], f32).T)
    shared["wo"] = np.ascontiguousarray(np.asarray(inputs["Wo"], f32).T)
    shared["wout"] = np.ascontiguousarray(np.asarray(inputs["W_out"], f32).T)
    shared["bout"] = np.ascontiguousarray(np.asarray(inputs["b_out"], f32)[None, :])

    in_maps = []
    for c in range(N_CORES):
        rows = slice(c * B_LOC, (c + 1) * B_LOC)
        m = dict(shared)
        m["xt"] = np.ascontiguousarray(xs[:, rows, :].transpose(0, 2, 1))
        in_maps.append(m)
    return in_maps


def _get_nc():
    if "nc" not in _CACHE:
        _CACHE["nc"] = build_nc()
    return _CACHE["nc"]


def kernel(**inputs):
    from concourse.bass_utils import run_bass_kernel_spmd

    nc = _get_nc()
    in_maps = _prep_inputs(inputs)
    res = run_bass_kernel_spmd(nc, in_maps, core_ids=list(range(N_CORES)))
    o = np.concatenate([res.results[c]["o_out"] for c in range(N_CORES)], axis=0)
    sm = np.concatenate([res.results[c]["sm_out"] for c in range(N_CORES)], axis=0)
    return o, sm
